# revision 9
# baseline (speedup 1.0000x reference)
"""Trainium2 Bass kernel for nn_Encoder_17978733101771 (2x ARMAConv + GroupNorm + tanh).

Sharding (8 cores): core c owns node-eighth c (10 windows x 128 slots,
bin-packed by in-degree); ALL 4 ARMA stacks live on every core, interleaved
along the feature dim of the gather tables (rows are [4*F] bf16 -> 1KB/2KB,
which halves SWDGE descriptor count vs per-stack rows).  Edges live with
their destination window, sorted by source, padded to a uniform
chunks-per-window (CPW).

Algebraic restructure: with dis[n] = rsqrt(max(deg,1)) masked, the ARMA
message pass agg[n] = sum_{e->n} norm_e * (state[src_e] + e_emb[e]) becomes
   agg[n] = dis[n] * ( seg_n(gather(dis*state)) + c[n] )
   c[n]   = A[n] @ ew + s[n] * eb,   A[n] = seg_n(dis[src] * edge_attr),
so the edge embedding never materializes per (t, stack) and A is shared by
both convs.

Device pipeline per core: A-phase -> c' -> per conv: S0 = x@iw -> scaled
bf16 table -> 8-core AllGather -> per window: dma_gather source rows
(4 SWDGE queues), one-hot Sel matmuls into PSUM (segment sum), epilogue
dis*seg + c' + x@rw + b, recurrence @w_arma -> next table; final t: local
mean over the 4 stacks, GroupNorm, tanh.
"""
import sys

sys.path.insert(0, "/opt/trn_rl_repo")

import heapq

import numpy as np
import ml_dtypes

# problem constants (hardcoded per contract)
N, E = 10000, 160000
F_IN, E_DIM, MID, OUT = 64, 16, 128, 256
K, T = 4, 2
GROUPS = 16
EPS = 1e-5

P = 128
NW = 10                 # windows per core
NC = 8
WTOT = NC * NW          # 80
NSLOT = NW * P          # 1280 node slots per core
RSLAB = NSLOT + 8       # table slab rows per rank (8 zero rows)

_BUILD_CACHE = {}


# ----------------------------------------------------------------------------
# Bass program
# ----------------------------------------------------------------------------
def _build_nc(CPW):
    import concourse.bacc as bacc
    import concourse.bass as bass
    import concourse.mybir as mybir
    import concourse.tile as tile
    from concourse import library_config

    f32 = mybir.dt.float32
    bf16 = mybir.dt.bfloat16
    i16 = mybir.dt.int16
    AF = mybir.ActivationFunctionType
    OP = mybir.AluOpType
    AX = mybir.AxisListType

    F1 = 4 * MID           # 512
    F2 = 4 * OUT           # 1024

    nc = bacc.Bacc("TRN2", num_devices=8, num_swdge_queues=4)

    def din(name, shape, dt=f32):
        return nc.dram_tensor(name, shape, dt, kind="ExternalInput")

    # ---- external inputs (per-core data)
    xT_d = din("xT", [F_IN, NSLOT])
    ea_d = din("ea", [P, NW, CPW, E_DIM])
    dsrc_d = din("dsrc", [P, NW, CPW])
    slot_d = din("slot", [P, NW, CPW])
    idx_d = din("idx", [P, NW * CPW * 8], i16)
    dcol_d = din("dcol", [P, NW])
    iota_d = din("iota", [P, P])
    ident_d = din("ident", [P, P])
    cw1_d = din("cw1", [E_DIM + 1, MID])
    cw2_d = din("cw2", [E_DIM + 1, OUT])
    wiw1_d = din("wiw1", [F_IN, F1])
    wrw1_d = din("wrw1", [F_IN, T, F1])
    b1_d = din("b1", [1, T * F1])
    wa1_d = din("wa1", [P, 4, MID], bf16)
    wiw2_d = din("wiw2", [MID, F2])
    wrw2_d = din("wrw2", [MID, T, F2])
    b2_d = din("b2", [1, T * F2])
    wa2_d = din("wa2", [P, 8, OUT], bf16)
    g1_d = din("g1", [P, MID])
    bt1_d = din("bt1", [P, MID])
    g2_d = din("g2", [P, OUT])
    bt2_d = din("bt2", [P, OUT])
    out_d = nc.dram_tensor("out", [NSLOT, OUT], f32, kind="ExternalOutput")

    # ---- internal DRAM
    tA1i = nc.dram_tensor("tA1i", [RSLAB, F1], bf16)
    tA1 = nc.dram_tensor("tA1", [8 * RSLAB, F1], bf16, addr_space="Shared")
    tB1i = nc.dram_tensor("tB1i", [RSLAB, F1], bf16)
    tB1 = nc.dram_tensor("tB1", [8 * RSLAB, F1], bf16, addr_space="Shared")
    tA2i = nc.dram_tensor("tA2i", [RSLAB, F2], bf16)
    tA2 = nc.dram_tensor("tA2", [8 * RSLAB, F2], bf16, addr_space="Shared")
    tB2i = nc.dram_tensor("tB2i", [RSLAB, F2], bf16)
    tB2 = nc.dram_tensor("tB2", [8 * RSLAB, F2], bf16, addr_space="Shared")

    ALL = [[0, 1, 2, 3, 4, 5, 6, 7]]

    nc.gpsimd.load_library(library_config.mlp)

    with tile.TileContext(nc) as tc:
        with (
            tc.tile_pool(name="const", bufs=1) as cp_,
            tc.tile_pool(name="work", bufs=2) as wk,
            tc.tile_pool(name="work1", bufs=1) as wk1,
            tc.tile_pool(name="psum1", bufs=1, space="PSUM") as ps1,
            tc.tile_pool(name="psum2", bufs=2, space="PSUM") as ps,
        ):
            # ---------- constants to SBUF
            def load_const(d, shape, dt=f32):
                t = cp_.tile(shape, dt, tag=f"c_{d.name}")
                nc.sync.dma_start(out=t[:], in_=d[:])
                return t

            xT_t = load_const(xT_d, [F_IN, NSLOT])
            dsrc_t = load_const(dsrc_d, [P, NW, CPW])
            slot_t = load_const(slot_d, [P, NW, CPW])
            idx_t = load_const(idx_d, [P, NW * CPW * 8], i16)
            dcol_t = load_const(dcol_d, [P, NW])
            iota_t = load_const(iota_d, [P, P])
            ident_t = load_const(ident_d, [P, P])
            cw1_t = load_const(cw1_d, [E_DIM + 1, MID])
            cw2_t = load_const(cw2_d, [E_DIM + 1, OUT])
            wiw1_t = load_const(wiw1_d, [F_IN, F1])
            wrw1_t = load_const(wrw1_d, [F_IN, T, F1])
            b1_t = load_const(b1_d, [1, T * F1])
            wa1_t = load_const(wa1_d, [P, 4, MID], bf16)
            wiw2_t = load_const(wiw2_d, [MID, F2])
            wrw2_t = load_const(wrw2_d, [MID, T, F2])
            b2_t = load_const(b2_d, [1, T * F2])
            wa2_t = load_const(wa2_d, [P, 8, OUT], bf16)
            g1_t = load_const(g1_d, [P, MID])
            bt1_t = load_const(bt1_d, [P, MID])
            g2_t = load_const(g2_d, [P, OUT])
            bt2_t = load_const(bt2_d, [P, OUT])

            ones1 = cp_.tile([1, P], f32, tag="ones1")
            nc.vector.memset(ones1[:], 1.0)
            eps_t = cp_.tile([P, 1], f32, tag="eps")
            nc.vector.memset(eps_t[:], EPS)

            # big residents
            AT_t = cp_.tile([32, NSLOT], f32, tag="AT")   # A'^T rows 0..16
            cpr_t = cp_.tile([P, NW, OUT], f32, tag="cpr")  # c' per conv
            hT_t = cp_.tile([MID, NSLOT], f32, tag="hT")    # conv2 xT

            # zero pad rows of the 4 table_in buffers
            zpad = cp_.tile([8, F2], bf16, tag="zpad")
            nc.vector.memset(zpad[:], 0)
            for tin, wd in ((tA1i, F1), (tB1i, F1), (tA2i, F2), (tB2i, F2)):
                nc.sync.dma_start(out=tin[NSLOT:RSLAB, :], in_=zpad[:, :wd])

            # ---------- helpers
            def sel_gen(w):
                sel = wk.tile([P, CPW, P], bf16, tag="sel")
                sl = slot_t[:, w, :]
                in0 = bass.AP(sl.tensor, sl.offset, [sl.ap[0], [1, CPW], [0, P]])
                io = iota_t[:]
                in1 = bass.AP(io.tensor, io.offset, [io.ap[0], [0, CPW], [1, P]])
                nc.vector.tensor_tensor(out=sel[:], in0=in0, in1=in1,
                                        op=OP.is_equal)
                return sel

            # ---------- A-phase: A' = dis_dst * seg(dis_src * [ea | 1])
            for w in range(NW):
                sel = sel_gen(w)
                eaw = wk.tile([P, CPW, E_DIM], f32, tag="eaw")
                nc.sync.dma_start(out=eaw[:], in_=ea_d[:, w, :, :])
                eam = wk.tile([P, CPW, E_DIM + 1], bf16, tag="eam")
                dsl = dsrc_t[:, w, :]
                dsb = bass.AP(dsl.tensor, dsl.offset,
                              [dsl.ap[0], [1, CPW], [0, E_DIM]])
                nc.vector.tensor_tensor(out=eam[:, :, :E_DIM], in0=eaw[:],
                                        in1=dsb, op=OP.mult)
                nc.vector.tensor_copy(out=eam[:, :, E_DIM:E_DIM + 1],
                                      in_=dsl[:, :, None])
                pA = ps1.tile([P, 32], f32, tag="pdpt", space="PSUM")
                for cc in range(CPW):
                    nc.tensor.matmul(out=pA[:, :E_DIM + 1],
                                     lhsT=sel[:, cc, :], rhs=eam[:, cc, :],
                                     start=(cc == 0), stop=(cc == CPW - 1))
                aq = wk.tile([P, 32], f32, tag="aq")
                nc.vector.memset(aq[:], 0)
                nc.vector.tensor_scalar_mul(aq[:, :E_DIM + 1],
                                            pA[:, :E_DIM + 1],
                                            dcol_t[:, w:w + 1])
                ptr = ps.tile([32, P], f32, tag="ptr", space="PSUM")
                nc.tensor.transpose(out=ptr[:], in_=aq[:], identity=ident_t[:])
                nc.vector.tensor_copy(out=AT_t[0:E_DIM + 1, w * P:(w + 1) * P],
                                      in_=ptr[0:E_DIM + 1, :])

            # ---------- one conv
            def conv(Fc, FW, Fin, xTsrc, cw_t, wiw_t, wrw_t, b_t, wa_t, nkt,
                     tAi, tA, tBi, tB, g_t, bt_t, final):
                NMM = FW // 512 if FW >= 512 else 1   # matmul col splits
                MMW = FW // NMM                        # cols per matmul
                # c' = A'@ew + s'*eb  (node-major f32, one stack's width)
                for w in range(NW):
                    pc = ps1.tile([P, FW], f32, tag="pdpt", space="PSUM")
                    nc.tensor.matmul(out=pc[:, :Fc],
                                     lhsT=AT_t[0:E_DIM + 1,
                                               w * P:(w + 1) * P],
                                     rhs=cw_t[:], start=True, stop=True)
                    nc.vector.tensor_copy(out=cpr_t[:, w, :Fc],
                                          in_=pc[:, :Fc])

                # S0 = x @ iw -> table A
                for w in range(NW):
                    pd = ps1.tile([P, FW], f32, tag="pdpt", space="PSUM")
                    for j in range(NMM):
                        nc.tensor.matmul(out=pd[:, j * MMW:(j + 1) * MMW],
                                         lhsT=xTsrc[:, w * P:(w + 1) * P],
                                         rhs=wiw_t[:, j * MMW:(j + 1) * MMW],
                                         start=True, stop=True)
                    tb = wk.tile([P, FW], bf16, tag="tb")
                    nc.scalar.activation(out=tb[:], in_=pd[:], func=AF.Copy,
                                         scale=dcol_t[:, w:w + 1])
                    nc.sync.dma_start(out=tAi[w * P:(w + 1) * P, :], in_=tb[:])
                nc.gpsimd.collective_compute("AllGather", OP.bypass,
                                             replica_groups=ALL,
                                             ins=[tAi[:]], outs=[tA[:]])

                for t in range(T):
                    tab = tA if t == 0 else tB
                    for w in range(NW):
                        sel = sel_gen(w)
                        msg = wk.tile([P, CPW, FW], bf16, tag="msg")
                        nsub = 4
                        step = (CPW + nsub - 1) // nsub
                        qn = 0
                        for a in range(0, CPW, step):
                            b = min(a + step, CPW)
                            nc.gpsimd.dma_gather(
                                msg[:, a:b, :], tab[:],
                                idx_t[:, (w * CPW + a) * 8:(w * CPW + b) * 8],
                                (b - a) * P, (b - a) * P, FW,
                                queue_num=qn % 4)
                            qn += 1
                        pseg = ps.tile([P, FW], f32, tag="pseg", space="PSUM")
                        for cc in range(CPW):
                            for j in range(NMM):
                                nc.tensor.matmul(
                                    out=pseg[:, j * MMW:(j + 1) * MMW],
                                    lhsT=sel[:, cc, :],
                                    rhs=msg[:, cc, j * MMW:(j + 1) * MMW],
                                    start=(cc == 0), stop=(cc == CPW - 1))
                        u = wk.tile([P, FW], f32, tag="u")
                        nc.scalar.activation(out=u[:], in_=pseg[:],
                                             func=AF.Copy,
                                             scale=dcol_t[:, w:w + 1])
                        pd = ps1.tile([P, FW], f32, tag="pdpt", space="PSUM")
                        for j in range(NMM):
                            nc.tensor.matmul(
                                out=pd[:, j * MMW:(j + 1) * MMW],
                                lhsT=xTsrc[:, w * P:(w + 1) * P],
                                rhs=wrw_t[:, t, j * MMW:(j + 1) * MMW],
                                start=True, stop=False)
                            nc.tensor.matmul(
                                out=pd[:, j * MMW:(j + 1) * MMW],
                                lhsT=ones1[:],
                                rhs=b_t[0:1, t * FW + j * MMW:
                                        t * FW + (j + 1) * MMW],
                                start=False, stop=True)
                        sa = wk1.tile([P, FW], f32, tag="sa")
                        for s in range(4):
                            nc.vector.tensor_tensor(
                                out=sa[:, s * Fc:(s + 1) * Fc],
                                in0=u[:, s * Fc:(s + 1) * Fc],
                                in1=cpr_t[:, w, :Fc], op=OP.add)
                        sb_ = wk1.tile([P, FW], f32, tag="sb")
                        nc.vector.tensor_tensor(out=sb_[:], in0=sa[:],
                                                in1=pd[:], op=OP.add)
                        if t < T - 1:
                            stt = wk.tile([P, FW // P, P], bf16, tag="stt")
                            for ft in range(FW // P):
                                ptr = ps.tile([P, P], f32, tag="ptr",
                                              space="PSUM")
                                nc.tensor.transpose(
                                    out=ptr[:],
                                    in_=sb_[:, ft * P:(ft + 1) * P],
                                    identity=ident_t[:])
                                nc.vector.tensor_copy(out=stt[:, ft, :],
                                                      in_=ptr[:])
                            pt = ps1.tile([P, FW], f32, tag="pdpt",
                                          space="PSUM")
                            for s in range(4):
                                for kt in range(nkt):
                                    nc.tensor.matmul(
                                        out=pt[:, s * Fc:(s + 1) * Fc],
                                        lhsT=stt[:, s * nkt + kt, :],
                                        rhs=wa_t[:, s * nkt + kt, :],
                                        start=(kt == 0), stop=(kt == nkt - 1))
                            tb = wk.tile([P, FW], bf16, tag="tb")
                            nc.scalar.activation(out=tb[:], in_=pt[:],
                                                 func=AF.Copy,
                                                 scale=dcol_t[:, w:w + 1])
                            nc.sync.dma_start(out=tBi[w * P:(w + 1) * P, :],
                                              in_=tb[:])
                        else:
                            # local mean over 4 stacks -> GroupNorm -> tanh
                            m = wk1.tile([P, Fc], f32, tag="mean")
                            nc.vector.tensor_reduce(
                                out=m[:],
                                in_=sb_[:].rearrange("p (s f) -> p f s", s=4),
                                axis=AX.X, op=OP.add)
                            m2 = wk1.tile([P, Fc], f32, tag="mean2")
                            nc.vector.tensor_scalar_mul(m2[:], m[:], 0.25)
                            gsz = Fc // GROUPS
                            mg = m2[:].rearrange("p (g s) -> p g s", g=GROUPS)
                            red = wk1.tile([P, GROUPS], f32, tag="red")
                            nc.vector.tensor_reduce(out=red[:], in_=mg,
                                                    axis=AX.X, op=OP.add)
                            sq = wk1.tile([P, Fc], f32, tag="sq")
                            nc.vector.tensor_tensor(out=sq[:], in0=m2[:],
                                                    in1=m2[:], op=OP.mult)
                            red2 = wk1.tile([P, GROUPS], f32, tag="red2")
                            nc.vector.tensor_reduce(
                                out=red2[:],
                                in_=sq[:].rearrange("p (g s) -> p g s",
                                                    g=GROUPS),
                                axis=AX.X, op=OP.add)
                            mu = wk1.tile([P, GROUPS], f32, tag="mu")
                            nc.vector.tensor_scalar_mul(mu[:], red[:],
                                                        1.0 / gsz)
                            ex2 = wk1.tile([P, GROUPS], f32, tag="ex2")
                            nc.vector.tensor_scalar_mul(ex2[:], red2[:],
                                                        1.0 / gsz)
                            mu2 = wk1.tile([P, GROUPS], f32, tag="mu2")
                            nc.vector.tensor_tensor(out=mu2[:], in0=mu[:],
                                                    in1=mu[:], op=OP.mult)
                            var = wk1.tile([P, GROUPS], f32, tag="var")
                            nc.vector.tensor_tensor(out=var[:], in0=ex2[:],
                                                    in1=mu2[:],
                                                    op=OP.subtract)
                            sd = wk1.tile([P, GROUPS], f32, tag="sd")
                            nc.scalar.activation(out=sd[:], in_=var[:],
                                                 func=AF.Sqrt, bias=eps_t[:])
                            rstd = wk1.tile([P, GROUPS], f32, tag="rstd")
                            nc.vector.reciprocal(out=rstd[:], in_=sd[:])
                            xc = wk1.tile([P, Fc], f32, tag="xc")
                            mua = mu[:]
                            mub = bass.AP(mua.tensor, mua.offset,
                                          [mua.ap[0], [1, GROUPS], [0, gsz]])
                            nc.vector.tensor_tensor(
                                out=xc[:].rearrange("p (g s) -> p g s",
                                                    g=GROUPS),
                                in0=mg, in1=mub, op=OP.subtract)
                            xn = wk1.tile([P, Fc], f32, tag="xn")
                            rsa = rstd[:]
                            rsb = bass.AP(rsa.tensor, rsa.offset,
                                          [rsa.ap[0], [1, GROUPS], [0, gsz]])
                            nc.vector.tensor_tensor(
                                out=xn[:].rearrange("p (g s) -> p g s",
                                                    g=GROUPS),
                                in0=xc[:].rearrange("p (g s) -> p g s",
                                                    g=GROUPS),
                                in1=rsb, op=OP.mult)
                            y1 = wk1.tile([P, Fc], f32, tag="y1")
                            nc.vector.tensor_tensor(out=y1[:], in0=xn[:],
                                                    in1=g_t[:], op=OP.mult)
                            y2 = wk1.tile([P, Fc], f32, tag="y2")
                            nc.vector.tensor_tensor(out=y2[:], in0=y1[:],
                                                    in1=bt_t[:], op=OP.add)
                            h = wk1.tile([P, Fc], f32, tag="h")
                            nc.scalar.activation(out=h[:], in_=y2[:],
                                                 func=AF.Tanh)
                            if final:
                                nc.sync.dma_start(
                                    out=out_d[w * P:(w + 1) * P, :],
                                    in_=h[:])
                            else:
                                ptr = ps.tile([P, P], f32, tag="ptr",
                                              space="PSUM")
                                nc.tensor.transpose(out=ptr[:], in_=h[:],
                                                    identity=ident_t[:])
                                nc.vector.tensor_copy(
                                    out=hT_t[:, w * P:(w + 1) * P],
                                    in_=ptr[:])
                    if t < T - 1:
                        nc.gpsimd.collective_compute("AllGather", OP.bypass,
                                                     replica_groups=ALL,
                                                     ins=[tBi[:]],
                                                     outs=[tB[:]])

            conv(MID, F1, F_IN, xT_t, cw1_t, wiw1_t, wrw1_t, b1_t,
                 wa1_t, 1, tA1i, tA1, tB1i, tB1, g1_t, bt1_t, False)
            conv(OUT, F2, MID, hT_t, cw2_t, wiw2_t, wrw2_t, b2_t,
                 wa2_t, 2, tA2i, tA2, tB2i, tB2, g2_t, bt2_t, True)

    nc.compile()
    return nc


# ----------------------------------------------------------------------------
# host preprocessing + run
# ----------------------------------------------------------------------------
def _pack_idxs(flat):
    """Pack flat gather indices (out position g = chunk*128 + partition)
    into the SWDGE dma_gather SBUF layout [128, nchunk*8] int16."""
    nchunk = len(flat) // P
    a = flat.reshape(nchunk, 8, 16)
    sb = np.transpose(a, (2, 0, 1)).reshape(16, nchunk * 8)
    return np.tile(sb, (8, 1)).astype(np.int16)


def kernel(**inputs):
    x = np.asarray(inputs["x"], np.float32)
    ea = np.asarray(inputs["edge_attr"], np.float32)
    ei = np.asarray(inputs["edge_index"])
    src = ei[:, 0].astype(np.int64)
    dst = ei[:, 1].astype(np.int64)

    deg = np.bincount(dst, minlength=N).astype(np.int64)
    dis = np.where(deg > 0, 1.0 / np.sqrt(np.maximum(deg, 1.0)), 0.0)
    dis = dis.astype(np.float32)

    # ---- bin-pack nodes into windows balancing in-degree
    order = np.argsort(-deg, kind="stable")
    heap = [(0, 0, w) for w in range(WTOT)]
    heapq.heapify(heap)
    win_of = np.empty(N, np.int32)
    slot_of = np.empty(N, np.int32)
    for n in order:
        while True:
            esum, cnt, w = heapq.heappop(heap)
            if cnt < P:
                break
        win_of[n] = w
        slot_of[n] = cnt
        heapq.heappush(heap, (esum + int(deg[n]), cnt + 1, w))
    core_of = win_of // NW
    wl_of = win_of % NW
    lrow = wl_of * P + slot_of              # [0, NSLOT) within core

    # ---- edges grouped by dst window, sorted by src
    ewin = win_of[dst]
    ord_e = np.lexsort((src, ewin))
    wcnt = np.bincount(ewin, minlength=WTOT)
    CPW = int(np.ceil(wcnt.max() / P))
    EPW = CPW * P
    starts = np.zeros(WTOT + 1, np.int64)
    np.cumsum(wcnt, out=starts[1:])

    nc = _BUILD_CACHE.get(CPW)
    if nc is None:
        nc = _build_nc(CPW)
        _BUILD_CACHE[CPW] = nc

    iota = np.tile(np.arange(P, dtype=np.float32)[None, :], (P, 1))
    ident = np.eye(P, dtype=np.float32)

    w1 = np.asarray(inputs["w1"], np.float32)
    w2 = np.asarray(inputs["w2"], np.float32)
    iw1 = np.asarray(inputs["iw1"], np.float32)
    iw2 = np.asarray(inputs["iw2"], np.float32)
    rw1 = np.asarray(inputs["rw1"], np.float32)
    rw2 = np.asarray(inputs["rw2"], np.float32)
    b1 = np.asarray(inputs["b1"], np.float32)
    b2 = np.asarray(inputs["b2"], np.float32)
    ew1 = np.asarray(inputs["ew1"], np.float32)
    ew2 = np.asarray(inputs["ew2"], np.float32)
    eb1 = np.asarray(inputs["eb1"], np.float32)
    eb2 = np.asarray(inputs["eb2"], np.float32)

    bf = ml_dtypes.bfloat16
    ksall = list(range(K))
    shared = {
        "iota": iota,
        "ident": ident,
        "cw1": np.concatenate([ew1, eb1[None, :]], 0),
        "cw2": np.concatenate([ew2, eb2[None, :]], 0),
        "wiw1": np.concatenate([iw1[k] for k in ksall], 1),
        "wrw1": np.stack(
            [np.concatenate([rw1[t, k] for k in ksall], 1)
             for t in range(T)], 1),
        "b1": np.concatenate(
            [np.concatenate([b1[t, k] for k in ksall])
             for t in range(T)])[None, :],
        "wa1": np.stack([w1[0, k] for k in ksall], 1).astype(bf),
        "wiw2": np.concatenate([iw2[k] for k in ksall], 1),
        "wrw2": np.stack(
            [np.concatenate([rw2[t, k] for k in ksall], 1)
             for t in range(T)], 1),
        "b2": np.concatenate(
            [np.concatenate([b2[t, k] for k in ksall])
             for t in range(T)])[None, :],
        "wa2": np.stack(
            [w2[0, k][kt * P:(kt + 1) * P, :]
             for k in ksall for kt in range(2)], 1).astype(bf),
        "g1": np.tile(np.asarray(inputs["gn1_g"], np.float32)[None, :],
                      (P, 1)),
        "bt1": np.tile(np.asarray(inputs["gn1_b"], np.float32)[None, :],
                       (P, 1)),
        "g2": np.tile(np.asarray(inputs["gn2_g"], np.float32)[None, :],
                      (P, 1)),
        "bt2": np.tile(np.asarray(inputs["gn2_b"], np.float32)[None, :],
                       (P, 1)),
    }

    row_of = core_of * RSLAB + lrow        # global table row per node
    zero_row = NSLOT                       # slab 0 pad row

    in_maps = []
    for c in range(NC):
        # per-window edge data
        idx_all = np.empty((NW, EPW), np.int64)
        slot_all = np.zeros((NW, EPW), np.float32)
        dsrc_all = np.zeros((NW, EPW), np.float32)
        ea_all = np.zeros((NW, EPW, E_DIM), np.float32)
        for wl in range(NW):
            w = c * NW + wl
            es = ord_e[starts[w]:starts[w + 1]]
            ne = len(es)
            idx_all[wl, :] = zero_row
            if ne:
                sr = src[es]
                idx_all[wl, :ne] = row_of[sr]
                slot_all[wl, :ne] = slot_of[dst[es]]
                dsrc_all[wl, :ne] = dis[sr]
                ea_all[wl, :ne, :] = ea[es]

        idx_packed = np.concatenate(
            [_pack_idxs(idx_all[wl]) for wl in range(NW)], axis=1)

        slot_a = slot_all.reshape(NW, CPW, P).transpose(2, 0, 1).copy()
        dsrc_a = dsrc_all.reshape(NW, CPW, P).transpose(2, 0, 1).copy()
        ea_a = (ea_all.reshape(NW, CPW, P, E_DIM)
                .transpose(2, 0, 1, 3).copy())

        cmask = core_of == c
        Xq = np.zeros((NSLOT, F_IN), np.float32)
        Xq[lrow[cmask]] = x[cmask]
        dcol = np.zeros((P, NW), np.float32)
        dcol[slot_of[cmask], wl_of[cmask]] = dis[cmask]

        in_maps.append(dict(shared,
                            xT=np.ascontiguousarray(Xq.T),
                            ea=ea_a, dsrc=dsrc_a, slot=slot_a,
                            idx=idx_packed, dcol=dcol))

    from concourse.bass_utils import run_bass_kernel_spmd
    res = run_bass_kernel_spmd(nc, in_maps, core_ids=list(range(8)))
    kernel._last_results = res

    full = np.zeros((N, OUT), np.float32)
    for c in range(NC):
        r = res.results[c]["out"]
        cmask = core_of == c
        full[cmask] = r[lrow[cmask]]
    return full
